# revision 62
# baseline (speedup 1.0000x reference)
"""DEC soft-assignment (student-t, row-normalized) Trainium2 Bass kernel.

q[n,k] = (1 + ||x_n - c_k||^2/alpha)^(-(alpha+1)/2), row-normalized.

Strategy (8 cores, data-parallel over N), v2 (~53-63 us/core steady state
vs the 149.6 us v1 baseline):
  ||x-c||^2 = ||x||^2 - 2 x.c + ||c||^2 expanded on-chip; the O(ND)
  norms and data-layout prep are done host-side so the device program is
  a pure fp8 DoubleRow matmul pipeline near the compute roofline:
  - host ships embT8 [D, N/8] fp8e4 (pre-transposed: d on partitions, so
    no PE transposes on device), cenT8 [D, K] fp8e4 scaled by -2/alpha,
    biasR = 1 + ||x||^2/alpha (f32, per-row ACT bias), and csq_bc
    [128, K] f32 = ||c||^2/alpha broadcast.
  - per 128-row block: PSUM <- 3 DoubleRow fp8 matmuls (256-contraction
    each, moving cenT 512 cols); DVE adds csq_bc (a PE fold matmul
    measured ~2x slower than this tensor_tensor add); ONE ACT Reciprocal
    computes numer = 1/(psum + biasR[p]) -> bf16 AND the row-sum via
    accum_out; DVE does 1/rowsum + normalize into a bf16 output tile.
  - output is bf16 (0.4% quantization << 2e-2 gate), halving write
    traffic; host upcasts to f32.  Measured rel err 1.05e-2.
  - measured engine budget per block: PE ~660ns (3 MMs), DVE ~925ns
    (add+recip+mul, the binding engine), ACT ~570ns, DMA ~45us total.
  KOPT_* env knobs switch the A/B variants explored during tuning (all
  defaults encode the fastest measured configuration).
"""

import contextlib
import os
import sys

sys.path.insert(0, "/opt/trn_rl_repo")

import numpy as np

N_CORES = 8
N, D, K = 65536, 768, 512
NC_ROWS = N // N_CORES          # 8192 rows per core
P = 128                         # partitions
S_BLK = int(os.environ.get("KOPT_SBLK", "8"))  # 128-row blocks per supertile
S_ROWS = P * S_BLK              # rows per supertile
N_SUPER = NC_ROWS // S_ROWS     # supertiles per core
NC_BLKS = NC_ROWS // P          # 64 blocks per core
D_CHUNKS = D // P               # 6 contraction chunks
G_BLK = 4                       # blocks per fold/recip group

_CACHE = {}


def _emit(nc, tc, embT_d, bias_d, cen_d, csq_d, csqbc_d, out_d, alpha: float, n_rows: int):
    """Emit the per-core program into an open TileContext."""
    import concourse.bass as bass
    import concourse.mybir as mybir

    f32 = mybir.dt.float32
    bf16 = mybir.dt.bfloat16
    fp8 = mybir.dt.float8e4
    DR = mybir.MatmulPerfMode.DoubleRow

    power = (alpha + 1.0) / 2.0
    n_super = n_rows // S_ROWS
    n_blks = n_rows // P
    reps = int(os.environ.get("KBENCH_REPS", "1"))
    # perturbation-profiling knobs (bottleneck attribution without NTFF):
    # comma-set of {mm,act,dve,out} to skip
    skip = set(filter(None, os.environ.get("KOPT_SKIP", "").split(",")))
    use_dr = os.environ.get("KOPT_DR", "1") == "1"
    fold_mode = os.environ.get("KOPT_FOLD", "dve")  # pe | dve | preload
    mul_mode = os.environ.get("KOPT_MUL", "dve")    # dve | gps | split
    g_blk = int(os.environ.get("KOPT_GBLK", str(G_BLK)))
    rs_mode = os.environ.get("KOPT_RS", "block")    # block | batch
    # csq as a full elementwise bias AP on the recip ACT (hw experiment);
    # xsq then moves to a 2x-mode tensor_scalar add on DVE
    act_bias = os.environ.get("KOPT_ACTB", "0") == "1"
    ps2 = os.environ.get("KOPT_PS2", "0") == "1"    # 2-bank psum pairs
    inv_eng = os.environ.get("KOPT_INV", "act")     # act | dve
    psum_bufs = max(6, g_blk)

    emb_v = embT_d.rearrange("(c p) (s n) -> s p c n", p=P, c=D_CHUNKS, n=S_ROWS)
    out_v = out_d.rearrange("(s a p) k -> s p a k", p=P, a=S_BLK)

    def act_recip(out_ap, in_ap, bias_ap, accum_ap):
        """numer = 1/(in + bias[p]); accum_out = row-sum(numer).
        Direct InstActivation(Reciprocal): measured max rel err ~1.2e-5
        on hw, well within this problem's 2e-2 gate."""
        eng = nc.scalar
        ins = [
            eng.lower_ap(in_ap),
            eng.lower_ap(bias_ap),
            mybir.ImmediateValue(dtype=f32, value=1.0),
            mybir.ImmediateValue(dtype=f32, value=0.0),
        ]
        outs = [eng.lower_ap(out_ap), eng.lower_ap(accum_ap)]
        return eng.add_instruction(
            mybir.InstActivation(
                name=nc.get_next_instruction_name(),
                func=mybir.ActivationFunctionType.Reciprocal,
                ins=ins,
                outs=outs,
            )
        )

    def act_recip_simple(out_ap, in_ap):
        """out = 1/in on the ACT engine (it has slack; DVE is binding)."""
        eng = nc.scalar
        ins = [
            eng.lower_ap(in_ap),
            mybir.ImmediateValue(dtype=f32, value=0.0),
            mybir.ImmediateValue(dtype=f32, value=1.0),
            mybir.ImmediateValue(dtype=f32, value=0.0),
        ]
        return eng.add_instruction(
            mybir.InstActivation(
                name=nc.get_next_instruction_name(),
                func=mybir.ActivationFunctionType.Reciprocal,
                ins=ins,
                outs=[eng.lower_ap(out_ap)],
            )
        )

    with contextlib.ExitStack() as stack:
        const_pool = stack.enter_context(tc.tile_pool(name="const", bufs=1))
        in_pool = stack.enter_context(tc.tile_pool(name="io_in", bufs=3))

        # resident operands (small DMAs, issued first)
        cenT = const_pool.tile([P, D_CHUNKS, K], fp8, name="cenT")
        csq8 = const_pool.tile([P, 2, K + P], fp8, name="csq8")
        biasR = const_pool.tile([P, n_blks], f32, name="biasR")
        csq_bc = const_pool.tile([P, K], f32, name="csq_bc")
        # setup loads ride the (idle) SWDGE queue so the first embedding
        # prefetches own the sync/HWDGE queue from instruction 0
        nc.gpsimd.dma_start(cenT[:], cen_d.rearrange("(c p) k -> p c k", p=P))
        nc.gpsimd.dma_start(csq8[:], csq_d[:])
        nc.gpsimd.dma_start(biasR[:], bias_d[:])
        if fold_mode in ("dve", "preload"):
            nc.gpsimd.dma_start(csq_bc[:], csqbc_d[:])
        # contiguous, aligned copies of the fold operands (the strided
        # [*,*,640] slices measure ~2x slower as matmul operands)
        ones_t = const_pool.tile([P, 2, P], fp8, name="ones_t")
        csq_mv_t = const_pool.tile([P, 2, K], fp8, name="csq_mv_t")
        nc.scalar.copy(ones_t[:], csq8[:, :, K : K + P])
        nc.scalar.copy(csq_mv_t[:], csq8[:, :, 0:K])
        ones_st = ones_t[:]     # DoubleRow fold stationary
        csq_mv = csq_mv_t[:]    # DoubleRow fold moving

        # prefetch the first supertiles' embeddings during setup; the very
        # first half-tile gets its own DMA so block 0 compute can start
        # after ~half the transfer time
        prefetched = {}
        for i in range(min(2, n_super)):
            t_in = in_pool.tile([P, D_CHUNKS, S_ROWS], fp8, tag="emb")
            if i == 0:
                half = S_ROWS // 2
                nc.sync.dma_start(t_in[:, :, 0:half], emb_v[i][:, :, 0:half])
                nc.sync.dma_start(t_in[:, :, half:], emb_v[i][:, :, half:])
            else:
                nc.sync.dma_start(t_in[:], emb_v[i])
            prefetched[i] = t_in

        # PE warmup: dense burst of dummy matmuls while the first DMAs
        # land, ramping the tensor engine's HAM clock-gate to full rate.
        n_warm = int(os.environ.get("KOPT_WARM", "24"))
        if n_warm:
            with (
                tc.tile_pool(name="warm", bufs=1) as warm_pool,
                tc.tile_pool(name="warm_ps", bufs=1, space=bass.MemorySpace.PSUM) as warm_ps_pool,
            ):
                wz = warm_pool.tile([P, P], f32)
                nc.gpsimd.memset(wz[:], 0.0)
                warm_ps = warm_ps_pool.tile([P, P], f32, tag="warm")
                for _ in range(n_warm):
                    nc.tensor.matmul(warm_ps[:], wz[:], wz[:], start=True, stop=True)

        ps_seq = [0]  # running count of ps allocations (first-use detection)
        with (
            tc.tile_pool(name="blk", bufs=8) as blk_pool,
            tc.tile_pool(name="io_out", bufs=2) as out_pool,
            tc.tile_pool(name="mm_ps", bufs=psum_bufs, space=bass.MemorySpace.PSUM) as mm_ps,
        ):
            for i in [t for _ in range(reps) for t in range(n_super)]:
                emb_t = prefetched.pop(i, None)
                if emb_t is None:
                    emb_t = in_pool.tile([P, D_CHUNKS, S_ROWS], fp8, tag="emb")
                    nc.sync.dma_start(emb_t[:], emb_v[i])

                out_t = out_pool.tile([P, S_BLK, K], bf16, tag="out")

                for g0 in range(0, S_BLK, g_blk):
                    blocks = range(g0, g0 + g_blk)
                    ps = {}
                    numers = {}
                    rs_g = None
                    # csq folds: ones stationary loaded once per group
                    skip_fold = ("mm" in skip or "fold" in skip
                                 or fold_mode not in ("pe", "pe_inline"))
                    skip_cross = "mm" in skip or "cross" in skip
                    preload = {}
                    pair_t = {}
                    for b in blocks:
                        if ps2:
                            if b % 2 == 0:
                                p2 = mm_ps.tile([P, 2, K], f32, tag="cross2",
                                                name=f"ps2_{b}", bufs=3)
                                pair_t[b] = pair_t[b + 1] = p2
                            ps[b] = pair_t[b][:, b % 2, :]
                        else:
                            t = mm_ps.tile([P, K], f32, tag="cross",
                                           name=f"ps{b}")
                            ps[b] = t[:]
                        # preload mode: after each PSUM buffer's first use,
                        # has_written is all-set, so a DVE copy of the csq row
                        # followed by start=False matmuls accumulates on top.
                        preload[b] = (
                            fold_mode == "preload"
                            and "mm" not in skip
                            and ps_seq[0] >= psum_bufs
                        )
                        ps_seq[0] += 1
                        if skip_fold:
                            if preload[b]:
                                nc.vector.tensor_copy(out=ps[b], in_=csq_bc[:])
                            continue
                        if fold_mode != "pe_inline":
                            nc.tensor.matmul(
                                ps[b], ones_st, csq_mv,
                                start=True, stop=skip_cross, perf_mode=DR,
                            )
                    for b in blocks:
                        # PSUM += -(2/a) x.c  (3 DoubleRow fp8 matmuls)
                        if fold_mode == "pe_inline" and not skip_fold:
                            nc.tensor.matmul(
                                ps[b], ones_st, csq_mv,
                                start=True, stop=skip_cross, perf_mode=DR,
                            )
                        if not skip_cross:
                            if use_dr:
                                for j in range(0, D_CHUNKS, 2):
                                    nc.tensor.matmul(
                                        ps[b],
                                        emb_t[:, j : j + 2, b * P : (b + 1) * P],
                                        cenT[:, j : j + 2, :],
                                        start=(skip_fold and j == 0
                                               and not preload[b]),
                                        stop=(j == D_CHUNKS - 2),
                                        perf_mode=DR,
                                        skip_group_check=preload[b],
                                    )
                            else:
                                for j in range(D_CHUNKS):
                                    nc.tensor.matmul(
                                        ps[b],
                                        emb_t[:, j, b * P : (b + 1) * P],
                                        cenT[:, j, :],
                                        start=(skip_fold and j == 0
                                               and not preload[b]),
                                        stop=(j == D_CHUNKS - 1),
                                        skip_group_check=preload[b],
                                    )
                        gb = i * S_BLK + b  # global block index (bias col)
                        if "act" in skip:
                            continue
                        do_fold_add = (
                            (fold_mode == "dve" or
                             (fold_mode == "preload" and not preload[b]))
                            and "mm" not in skip
                        )
                        todo = [b]
                        if ps2 and power == 1.0 and rs_mode == "block":
                            if b % 2 == 0:
                                continue  # processed with the odd partner
                            if do_fold_add:
                                nc.vector.tensor_tensor(
                                    pair_t[b][:, :, :], pair_t[b][:, :, :],
                                    csq_bc[:, None, :].to_broadcast([P, 2, K]),
                                    mybir.AluOpType.add,
                                )
                            todo = [b - 1, b]
                        elif do_fold_add:
                            if act_bias:
                                # xsq via 2x-mode tensor_scalar (csq rides the
                                # ACT bias slot elementwise)
                                nc.vector.tensor_scalar_add(
                                    ps[b], ps[b],
                                    biasR[:, gb : gb + 1],
                                )
                            else:
                                # csq add on DVE instead of the PE fold matmul
                                nc.vector.tensor_tensor(
                                    ps[b], ps[b], csq_bc[:],
                                    mybir.AluOpType.add,
                                )
                        if power == 1.0:
                            for bb in todo:
                                gbb = i * S_BLK + bb
                                # numer = 1/(ps + bias) -> bf16 + rowsum, one ACT
                                numer = blk_pool.tile([P, K], bf16, tag="numer",
                                                      bufs=g_blk + 4)
                                bias_ap = (csq_bc[:] if act_bias
                                           else biasR[:, gbb : gbb + 1])
                                if rs_mode == "batch":
                                    if rs_g is None:
                                        rs_g = blk_pool.tile([P, g_blk], f32,
                                                             tag="rs")
                                    act_recip(
                                        numer[:], ps[bb], bias_ap,
                                        rs_g[:, bb - g0 : bb - g0 + 1],
                                    )
                                    numers[bb] = numer
                                else:
                                    rs = blk_pool.tile([P, 1], f32, tag="rs1")
                                    act_recip(
                                        numer[:], ps[bb], bias_ap,
                                        rs[:],
                                    )
                                    if "dve" in skip:
                                        continue
                                    inv = blk_pool.tile([P, 1], f32, tag="inv1")
                                    if inv_eng == "act":
                                        act_recip_simple(inv[:], rs[:])
                                    else:
                                        nc.vector.reciprocal(inv[:], rs[:])
                                    nc.vector.tensor_scalar_mul(
                                        out_t[:, bb, :], numer[:], inv[:]
                                    )
                        else:
                            denom = blk_pool.tile([P, K], f32, tag="denom")
                            nc.scalar.activation(
                                denom[:], ps[b],
                                mybir.ActivationFunctionType.Identity,
                                bias=biasR[:, gb : gb + 1], scale=1.0,
                            )
                            lnd = blk_pool.tile([P, K], f32, tag="lnd")
                            nc.scalar.activation(
                                lnd[:], denom[:], mybir.ActivationFunctionType.Ln
                            )
                            numer = blk_pool.tile([P, K], bf16, tag="numer")
                            rs = blk_pool.tile([P, 1], f32, tag="rs")
                            nc.scalar.activation(
                                numer[:], lnd[:],
                                mybir.ActivationFunctionType.Exp,
                                scale=-power, accum_out=rs[:],
                            )
                            inv = blk_pool.tile([P, 1], f32, tag="inv")
                            nc.vector.reciprocal(inv[:], rs[:])
                            nc.vector.tensor_scalar_mul(
                                out_t[:, b, :], numer[:], inv[:]
                            )

                    if power == 1.0 and numers and "dve" not in skip:
                        # batched 1/rowsum for the group, then normalize
                        inv_g = blk_pool.tile([P, g_blk], f32, tag="inv")
                        nc.vector.reciprocal(inv_g[:], rs_g[:])
                        for b in blocks:
                            if b not in numers:
                                continue
                            sc = inv_g[:, b - g0 : b - g0 + 1]
                            if mul_mode == "split" and (b % 2 == 1):
                                nc.scalar.activation(
                                    out_t[:, b, :], numers[b][:],
                                    mybir.ActivationFunctionType.Copy,
                                    scale=sc,
                                )
                            elif mul_mode == "gps":
                                nc.gpsimd.tensor_scalar_mul(
                                    out_t[:, b, :], numers[b][:], sc
                                )
                            else:
                                nc.vector.tensor_scalar_mul(
                                    out_t[:, b, :], numers[b][:], sc
                                )

                # output triggers on the (otherwise idle) GpSimd queue; two
                # half-supertile DMAs so the first half ships while the
                # second half computes (subtile deps), shortening the tail
                if "out" not in skip:
                    h = S_BLK // 2
                    nc.gpsimd.dma_start(out_v[i][:, 0:h, :], out_t[:, 0:h, :])
                    nc.gpsimd.dma_start(out_v[i][:, h:, :], out_t[:, h:, :])


def _host_prep(embeddings, cluster_centers, alpha: float):
    """Layout/precision prep: transpose+fp8-cast emb, row norms, center
    norms (hi/lo fp8 split), -2/alpha-scaled fp8 centers-T."""
    import ml_dtypes

    fp8 = ml_dtypes.float8_e4m3
    bf16 = ml_dtypes.bfloat16  # noqa: F841  (output dtype, upcast in kernel())

    emb = np.ascontiguousarray(np.asarray(embeddings, dtype=np.float32))
    cen = np.ascontiguousarray(np.asarray(cluster_centers, dtype=np.float32))
    inv_a = 1.0 / alpha

    embT8 = np.ascontiguousarray(emb.astype(fp8).T)              # [D, N]
    xsq = np.einsum("nd,nd->n", emb, emb, dtype=np.float32)
    biasR = np.ascontiguousarray(
        (1.0 + xsq * inv_a).astype(np.float32).reshape(N // P, P).T
    )                                                            # [P, N/P]
    cenT8 = np.ascontiguousarray((cen.T * np.float32(-2.0 * inv_a)).astype(fp8))
    csq = np.einsum("kd,kd->k", cen, cen, dtype=np.float32)
    tgt = (csq * np.float32(inv_a / P)).astype(np.float32)       # [K]
    hi = tgt.astype(fp8)
    lo = (tgt - hi.astype(np.float32)).astype(fp8)
    csq8 = np.empty((P, 2, K + P), dtype=fp8)
    csq8[:, 0, :K] = hi[None, :]
    csq8[:, 1, :K] = lo[None, :]
    csq8[:, :, K:] = np.float32(1.0).astype(fp8)
    csq_bc = np.ascontiguousarray(
        np.broadcast_to((csq * np.float32(inv_a)).astype(np.float32)[None, :], (P, K))
    )
    return embT8, biasR, cenT8, csq8, csq_bc


def _get_jitted(alpha: float):
    key = (float(alpha), os.environ.get("KBENCH_REPS", "1"),
           os.environ.get("KOPT_WARM", "24"),
           os.environ.get("KOPT_SKIP", ""), os.environ.get("KOPT_DR", "1"),
           os.environ.get("KOPT_FOLD", "dve"), os.environ.get("KOPT_MUL", "dve"),
           os.environ.get("KOPT_GBLK", str(G_BLK)),
           os.environ.get("KOPT_RS", "block"),
           os.environ.get("KOPT_ACTB", "0"), str(S_BLK),
           os.environ.get("KOPT_PS2", "0"), os.environ.get("KOPT_INV", "act"))
    if key in _CACHE:
        return _CACHE[key]

    import jax
    from jax.experimental.shard_map import shard_map
    from jax.sharding import Mesh, PartitionSpec as PS

    import concourse.mybir as mybir
    import concourse.tile as tile
    from concourse.bass2jax import bass_jit

    in_specs = (PS(None, "core"), PS(None, "core"), PS(None), PS(None), PS(None))

    bf16 = mybir.dt.bfloat16

    def body(nc, embT8, biasR, cenT8, csq8, csq_bc):
        out_d = nc.dram_tensor(
            "cluster_p", [NC_ROWS, K], bf16, kind="ExternalOutput"
        )
        with tile.TileContext(nc) as tc:
            _emit(nc, tc, embT8, biasR, cenT8, csq8, csq_bc, out_d,
                  float(alpha), NC_ROWS)
        return out_d

    f = bass_jit(body, num_devices=N_CORES)
    mesh = Mesh(np.asarray(jax.devices()[:N_CORES]), ("core",))
    sharded = shard_map(
        f,
        mesh=mesh,
        in_specs=in_specs,
        out_specs=PS("core"),
        check_rep=False,
    )
    jitted = jax.jit(sharded)
    _CACHE[key] = (jitted, mesh)
    return _CACHE[key]


def kernel(embeddings, cluster_centers, alpha):
    alpha = float(alpha)
    args = _host_prep(embeddings, cluster_centers, alpha)
    jitted, _ = _get_jitted(alpha)
    try:
        out = jitted(*args)
        return np.asarray(out).astype(np.float32)
    except Exception:
        # transient device hiccups have been observed; retry once
        import time as _time

        _time.sleep(60)
        out = jitted(*args)
        return np.asarray(out).astype(np.float32)


def _build_program(alpha: float):
    """Standalone Bacc program (for CoreSim checks)."""
    import concourse.bacc as bacc
    import concourse.mybir as mybir
    import concourse.tile as tile

    f32 = mybir.dt.float32
    fp8 = mybir.dt.float8e4
    bf16 = mybir.dt.bfloat16
    nc = bacc.Bacc(None, target_bir_lowering=False, debug=False, num_devices=N_CORES)
    embT_d = nc.declare_dram_parameter("embT8", [D, NC_ROWS], fp8, isOutput=False)
    bias_d = nc.declare_dram_parameter("biasR", [P, NC_BLKS], f32, isOutput=False)
    cen_d = nc.declare_dram_parameter("cenT8", [D, K], fp8, isOutput=False)
    csq_d = nc.declare_dram_parameter("csq8", [P, 2, K + P], fp8, isOutput=False)
    csqbc_d = nc.declare_dram_parameter("csq_bc", [P, K], f32, isOutput=False)
    out_d = nc.declare_dram_parameter("cluster_p", [NC_ROWS, K], bf16, isOutput=True)
    with tile.TileContext(nc) as tc:
        _emit(nc, tc, embT_d, bias_d, cen_d, csq_d, csqbc_d, out_d, alpha, NC_ROWS)
    nc.finalize()
    return nc


# revision 64
# speedup vs baseline: 1.0056x; 1.0056x over previous
"""DEC soft-assignment (student-t, row-normalized) Trainium2 Bass kernel.

q[n,k] = (1 + ||x_n - c_k||^2/alpha)^(-(alpha+1)/2), row-normalized.

Strategy (8 cores, data-parallel over N), v2 (~53-63 us/core steady state
vs the 149.6 us v1 baseline):
  ||x-c||^2 = ||x||^2 - 2 x.c + ||c||^2 expanded on-chip; the O(ND)
  norms and data-layout prep are done host-side so the device program is
  a pure fp8 DoubleRow matmul pipeline near the compute roofline:
  - host ships embT8 [D, N/8] fp8e4 (pre-transposed: d on partitions, so
    no PE transposes on device), cenT8 [D, K] fp8e4 scaled by -2/alpha,
    biasR = 1 + ||x||^2/alpha (f32, per-row ACT bias), and csq_bc
    [128, K] f32 = ||c||^2/alpha broadcast.
  - per 128-row block: PSUM <- 3 DoubleRow fp8 matmuls (256-contraction
    each, moving cenT 512 cols); DVE adds csq_bc (a PE fold matmul
    measured ~2x slower than this tensor_tensor add); ONE ACT Reciprocal
    computes numer = 1/(psum + biasR[p]) -> bf16 AND the row-sum via
    accum_out; DVE does 1/rowsum + normalize into a bf16 output tile.
  - output is bf16 (0.4% quantization << 2e-2 gate), halving write
    traffic; host upcasts to f32.  Measured rel err 1.05e-2.
  - measured engine budget per block: PE ~660ns (3 MMs), DVE ~925ns
    (add+recip+mul, the binding engine), ACT ~570ns, DMA ~45us total.
  KOPT_* env knobs switch the A/B variants explored during tuning (all
  defaults encode the fastest measured configuration).
"""

import contextlib
import os
import sys

sys.path.insert(0, "/opt/trn_rl_repo")

import numpy as np

N_CORES = 8
N, D, K = 65536, 768, 512
NC_ROWS = N // N_CORES          # 8192 rows per core
P = 128                         # partitions
S_BLK = int(os.environ.get("KOPT_SBLK", "8"))  # 128-row blocks per supertile
S_ROWS = P * S_BLK              # rows per supertile
N_SUPER = NC_ROWS // S_ROWS     # supertiles per core
NC_BLKS = NC_ROWS // P          # 64 blocks per core
D_CHUNKS = D // P               # 6 contraction chunks
G_BLK = 4                       # blocks per fold/recip group

_CACHE = {}


def _emit(nc, tc, embT_d, bias_d, cen_d, csq_d, csqbc_d, out_d, alpha: float, n_rows: int):
    """Emit the per-core program into an open TileContext."""
    import concourse.bass as bass
    import concourse.mybir as mybir

    f32 = mybir.dt.float32
    bf16 = mybir.dt.bfloat16
    fp8 = mybir.dt.float8e4
    DR = mybir.MatmulPerfMode.DoubleRow

    power = (alpha + 1.0) / 2.0
    n_super = n_rows // S_ROWS
    n_blks = n_rows // P
    reps = int(os.environ.get("KBENCH_REPS", "1"))
    # perturbation-profiling knobs (bottleneck attribution without NTFF):
    # comma-set of {mm,act,dve,out} to skip
    skip = set(filter(None, os.environ.get("KOPT_SKIP", "").split(",")))
    use_dr = os.environ.get("KOPT_DR", "1") == "1"
    fold_mode = os.environ.get("KOPT_FOLD", "dve")  # pe | dve | preload
    mul_mode = os.environ.get("KOPT_MUL", "dve")    # dve | gps | split
    g_blk = int(os.environ.get("KOPT_GBLK", str(G_BLK)))
    rs_mode = os.environ.get("KOPT_RS", "block")    # block | batch
    # csq as a full elementwise bias AP on the recip ACT (hw experiment);
    # xsq then moves to a 2x-mode tensor_scalar add on DVE
    act_bias = os.environ.get("KOPT_ACTB", "0") == "1"
    ps2 = os.environ.get("KOPT_PS2", "0") == "1"    # 2-bank psum pairs
    inv_eng = os.environ.get("KOPT_INV", "dve")     # dve | act
    psum_bufs = max(6, g_blk)

    emb_v = embT_d.rearrange("(c p) (s n) -> s p c n", p=P, c=D_CHUNKS, n=S_ROWS)
    out_v = out_d.rearrange("(s a p) k -> s p a k", p=P, a=S_BLK)

    def act_recip(out_ap, in_ap, bias_ap, accum_ap):
        """numer = 1/(in + bias[p]); accum_out = row-sum(numer).
        Direct InstActivation(Reciprocal): measured max rel err ~1.2e-5
        on hw, well within this problem's 2e-2 gate."""
        eng = nc.scalar
        ins = [
            eng.lower_ap(in_ap),
            eng.lower_ap(bias_ap),
            mybir.ImmediateValue(dtype=f32, value=1.0),
            mybir.ImmediateValue(dtype=f32, value=0.0),
        ]
        outs = [eng.lower_ap(out_ap), eng.lower_ap(accum_ap)]
        return eng.add_instruction(
            mybir.InstActivation(
                name=nc.get_next_instruction_name(),
                func=mybir.ActivationFunctionType.Reciprocal,
                ins=ins,
                outs=outs,
            )
        )

    def act_recip_simple(out_ap, in_ap):
        """out = 1/in on the ACT engine (it has slack; DVE is binding)."""
        eng = nc.scalar
        ins = [
            eng.lower_ap(in_ap),
            mybir.ImmediateValue(dtype=f32, value=0.0),
            mybir.ImmediateValue(dtype=f32, value=1.0),
            mybir.ImmediateValue(dtype=f32, value=0.0),
        ]
        return eng.add_instruction(
            mybir.InstActivation(
                name=nc.get_next_instruction_name(),
                func=mybir.ActivationFunctionType.Reciprocal,
                ins=ins,
                outs=[eng.lower_ap(out_ap)],
            )
        )

    with contextlib.ExitStack() as stack:
        const_pool = stack.enter_context(tc.tile_pool(name="const", bufs=1))
        in_pool = stack.enter_context(tc.tile_pool(name="io_in", bufs=3))

        # resident operands (small DMAs, issued first)
        cenT = const_pool.tile([P, D_CHUNKS, K], fp8, name="cenT")
        csq8 = const_pool.tile([P, 2, K + P], fp8, name="csq8")
        biasR = const_pool.tile([P, n_blks], f32, name="biasR")
        csq_bc = const_pool.tile([P, K], f32, name="csq_bc")
        # setup loads ride the (idle) SWDGE queue so the first embedding
        # prefetches own the sync/HWDGE queue from instruction 0
        nc.gpsimd.dma_start(cenT[:], cen_d.rearrange("(c p) k -> p c k", p=P))
        nc.gpsimd.dma_start(csq8[:], csq_d[:])
        nc.gpsimd.dma_start(biasR[:], bias_d[:])
        if fold_mode in ("dve", "preload"):
            nc.gpsimd.dma_start(csq_bc[:], csqbc_d[:])
        # contiguous, aligned copies of the fold operands (the strided
        # [*,*,640] slices measure ~2x slower as matmul operands)
        ones_t = const_pool.tile([P, 2, P], fp8, name="ones_t")
        csq_mv_t = const_pool.tile([P, 2, K], fp8, name="csq_mv_t")
        nc.scalar.copy(ones_t[:], csq8[:, :, K : K + P])
        nc.scalar.copy(csq_mv_t[:], csq8[:, :, 0:K])
        ones_st = ones_t[:]     # DoubleRow fold stationary
        csq_mv = csq_mv_t[:]    # DoubleRow fold moving

        # prefetch the first supertiles' embeddings during setup; the very
        # first half-tile gets its own DMA so block 0 compute can start
        # after ~half the transfer time
        prefetched = {}
        for i in range(min(2, n_super)):
            t_in = in_pool.tile([P, D_CHUNKS, S_ROWS], fp8, tag="emb")
            if i == 0:
                half = S_ROWS // 2
                nc.sync.dma_start(t_in[:, :, 0:half], emb_v[i][:, :, 0:half])
                nc.sync.dma_start(t_in[:, :, half:], emb_v[i][:, :, half:])
            else:
                nc.sync.dma_start(t_in[:], emb_v[i])
            prefetched[i] = t_in

        # PE warmup: dense burst of dummy matmuls while the first DMAs
        # land, ramping the tensor engine's HAM clock-gate to full rate.
        n_warm = int(os.environ.get("KOPT_WARM", "24"))
        if n_warm:
            with (
                tc.tile_pool(name="warm", bufs=1) as warm_pool,
                tc.tile_pool(name="warm_ps", bufs=1, space=bass.MemorySpace.PSUM) as warm_ps_pool,
            ):
                wz = warm_pool.tile([P, P], f32)
                nc.gpsimd.memset(wz[:], 0.0)
                warm_ps = warm_ps_pool.tile([P, P], f32, tag="warm")
                for _ in range(n_warm):
                    nc.tensor.matmul(warm_ps[:], wz[:], wz[:], start=True, stop=True)

        ps_seq = [0]  # running count of ps allocations (first-use detection)
        with (
            tc.tile_pool(name="blk", bufs=8) as blk_pool,
            tc.tile_pool(name="io_out", bufs=2) as out_pool,
            tc.tile_pool(name="mm_ps", bufs=psum_bufs, space=bass.MemorySpace.PSUM) as mm_ps,
        ):
            for i in [t for _ in range(reps) for t in range(n_super)]:
                emb_t = prefetched.pop(i, None)
                if emb_t is None:
                    emb_t = in_pool.tile([P, D_CHUNKS, S_ROWS], fp8, tag="emb")
                    nc.sync.dma_start(emb_t[:], emb_v[i])

                out_t = out_pool.tile([P, S_BLK, K], bf16, tag="out")

                for g0 in range(0, S_BLK, g_blk):
                    blocks = range(g0, g0 + g_blk)
                    ps = {}
                    numers = {}
                    rs_g = None
                    # csq folds: ones stationary loaded once per group
                    skip_fold = ("mm" in skip or "fold" in skip
                                 or fold_mode not in ("pe", "pe_inline"))
                    skip_cross = "mm" in skip or "cross" in skip
                    preload = {}
                    pair_t = {}
                    for b in blocks:
                        if ps2:
                            if b % 2 == 0:
                                p2 = mm_ps.tile([P, 2, K], f32, tag="cross2",
                                                name=f"ps2_{b}", bufs=3)
                                pair_t[b] = pair_t[b + 1] = p2
                            ps[b] = pair_t[b][:, b % 2, :]
                        else:
                            t = mm_ps.tile([P, K], f32, tag="cross",
                                           name=f"ps{b}")
                            ps[b] = t[:]
                        # preload mode: after each PSUM buffer's first use,
                        # has_written is all-set, so a DVE copy of the csq row
                        # followed by start=False matmuls accumulates on top.
                        preload[b] = (
                            fold_mode == "preload"
                            and "mm" not in skip
                            and ps_seq[0] >= psum_bufs
                        )
                        ps_seq[0] += 1
                        if skip_fold:
                            if preload[b]:
                                nc.vector.tensor_copy(out=ps[b], in_=csq_bc[:])
                            continue
                        if fold_mode != "pe_inline":
                            nc.tensor.matmul(
                                ps[b], ones_st, csq_mv,
                                start=True, stop=skip_cross, perf_mode=DR,
                            )
                    for b in blocks:
                        # PSUM += -(2/a) x.c  (3 DoubleRow fp8 matmuls)
                        if fold_mode == "pe_inline" and not skip_fold:
                            nc.tensor.matmul(
                                ps[b], ones_st, csq_mv,
                                start=True, stop=skip_cross, perf_mode=DR,
                            )
                        if not skip_cross:
                            if use_dr:
                                for j in range(0, D_CHUNKS, 2):
                                    nc.tensor.matmul(
                                        ps[b],
                                        emb_t[:, j : j + 2, b * P : (b + 1) * P],
                                        cenT[:, j : j + 2, :],
                                        start=(skip_fold and j == 0
                                               and not preload[b]),
                                        stop=(j == D_CHUNKS - 2),
                                        perf_mode=DR,
                                        skip_group_check=preload[b],
                                    )
                            else:
                                for j in range(D_CHUNKS):
                                    nc.tensor.matmul(
                                        ps[b],
                                        emb_t[:, j, b * P : (b + 1) * P],
                                        cenT[:, j, :],
                                        start=(skip_fold and j == 0
                                               and not preload[b]),
                                        stop=(j == D_CHUNKS - 1),
                                        skip_group_check=preload[b],
                                    )
                        gb = i * S_BLK + b  # global block index (bias col)
                        if "act" in skip:
                            continue
                        do_fold_add = (
                            (fold_mode == "dve" or
                             (fold_mode == "preload" and not preload[b]))
                            and "mm" not in skip
                        )
                        todo = [b]
                        if ps2 and power == 1.0 and rs_mode == "block":
                            if b % 2 == 0:
                                continue  # processed with the odd partner
                            if do_fold_add:
                                nc.vector.tensor_tensor(
                                    pair_t[b][:, :, :], pair_t[b][:, :, :],
                                    csq_bc[:, None, :].to_broadcast([P, 2, K]),
                                    mybir.AluOpType.add,
                                )
                            todo = [b - 1, b]
                        elif do_fold_add:
                            if act_bias:
                                # xsq via 2x-mode tensor_scalar (csq rides the
                                # ACT bias slot elementwise)
                                nc.vector.tensor_scalar_add(
                                    ps[b], ps[b],
                                    biasR[:, gb : gb + 1],
                                )
                            else:
                                # csq add on DVE instead of the PE fold matmul
                                nc.vector.tensor_tensor(
                                    ps[b], ps[b], csq_bc[:],
                                    mybir.AluOpType.add,
                                )
                        if power == 1.0:
                            for bb in todo:
                                gbb = i * S_BLK + bb
                                # numer = 1/(ps + bias) -> bf16 + rowsum, one ACT
                                numer = blk_pool.tile([P, K], bf16, tag="numer",
                                                      bufs=g_blk + 4)
                                bias_ap = (csq_bc[:] if act_bias
                                           else biasR[:, gbb : gbb + 1])
                                if rs_mode == "batch":
                                    if rs_g is None:
                                        rs_g = blk_pool.tile([P, g_blk], f32,
                                                             tag="rs")
                                    act_recip(
                                        numer[:], ps[bb], bias_ap,
                                        rs_g[:, bb - g0 : bb - g0 + 1],
                                    )
                                    numers[bb] = numer
                                else:
                                    rs = blk_pool.tile([P, 1], f32, tag="rs1")
                                    act_recip(
                                        numer[:], ps[bb], bias_ap,
                                        rs[:],
                                    )
                                    if "dve" in skip:
                                        continue
                                    inv = blk_pool.tile([P, 1], f32, tag="inv1")
                                    if inv_eng == "act":
                                        act_recip_simple(inv[:], rs[:])
                                    else:
                                        nc.vector.reciprocal(inv[:], rs[:])
                                    nc.vector.tensor_scalar_mul(
                                        out_t[:, bb, :], numer[:], inv[:]
                                    )
                        else:
                            denom = blk_pool.tile([P, K], f32, tag="denom")
                            nc.scalar.activation(
                                denom[:], ps[b],
                                mybir.ActivationFunctionType.Identity,
                                bias=biasR[:, gb : gb + 1], scale=1.0,
                            )
                            lnd = blk_pool.tile([P, K], f32, tag="lnd")
                            nc.scalar.activation(
                                lnd[:], denom[:], mybir.ActivationFunctionType.Ln
                            )
                            numer = blk_pool.tile([P, K], bf16, tag="numer")
                            rs = blk_pool.tile([P, 1], f32, tag="rs")
                            nc.scalar.activation(
                                numer[:], lnd[:],
                                mybir.ActivationFunctionType.Exp,
                                scale=-power, accum_out=rs[:],
                            )
                            inv = blk_pool.tile([P, 1], f32, tag="inv")
                            nc.vector.reciprocal(inv[:], rs[:])
                            nc.vector.tensor_scalar_mul(
                                out_t[:, b, :], numer[:], inv[:]
                            )

                    if power == 1.0 and numers and "dve" not in skip:
                        # batched 1/rowsum for the group, then normalize
                        inv_g = blk_pool.tile([P, g_blk], f32, tag="inv")
                        nc.vector.reciprocal(inv_g[:], rs_g[:])
                        for b in blocks:
                            if b not in numers:
                                continue
                            sc = inv_g[:, b - g0 : b - g0 + 1]
                            if mul_mode == "split" and (b % 2 == 1):
                                nc.scalar.activation(
                                    out_t[:, b, :], numers[b][:],
                                    mybir.ActivationFunctionType.Copy,
                                    scale=sc,
                                )
                            elif mul_mode == "gps":
                                nc.gpsimd.tensor_scalar_mul(
                                    out_t[:, b, :], numers[b][:], sc
                                )
                            else:
                                nc.vector.tensor_scalar_mul(
                                    out_t[:, b, :], numers[b][:], sc
                                )

                # output triggers on the (otherwise idle) GpSimd queue; two
                # half-supertile DMAs so the first half ships while the
                # second half computes (subtile deps), shortening the tail
                if "out" not in skip:
                    h = S_BLK // 2
                    nc.gpsimd.dma_start(out_v[i][:, 0:h, :], out_t[:, 0:h, :])
                    nc.gpsimd.dma_start(out_v[i][:, h:, :], out_t[:, h:, :])


def _host_prep(embeddings, cluster_centers, alpha: float):
    """Layout/precision prep: transpose+fp8-cast emb, row norms, center
    norms (hi/lo fp8 split), -2/alpha-scaled fp8 centers-T."""
    import ml_dtypes

    fp8 = ml_dtypes.float8_e4m3
    bf16 = ml_dtypes.bfloat16  # noqa: F841  (output dtype, upcast in kernel())

    emb = np.ascontiguousarray(np.asarray(embeddings, dtype=np.float32))
    cen = np.ascontiguousarray(np.asarray(cluster_centers, dtype=np.float32))
    inv_a = 1.0 / alpha

    embT8 = np.ascontiguousarray(emb.astype(fp8).T)              # [D, N]
    xsq = np.einsum("nd,nd->n", emb, emb, dtype=np.float32)
    biasR = np.ascontiguousarray(
        (1.0 + xsq * inv_a).astype(np.float32).reshape(N // P, P).T
    )                                                            # [P, N/P]
    cenT8 = np.ascontiguousarray((cen.T * np.float32(-2.0 * inv_a)).astype(fp8))
    csq = np.einsum("kd,kd->k", cen, cen, dtype=np.float32)
    tgt = (csq * np.float32(inv_a / P)).astype(np.float32)       # [K]
    hi = tgt.astype(fp8)
    lo = (tgt - hi.astype(np.float32)).astype(fp8)
    csq8 = np.empty((P, 2, K + P), dtype=fp8)
    csq8[:, 0, :K] = hi[None, :]
    csq8[:, 1, :K] = lo[None, :]
    csq8[:, :, K:] = np.float32(1.0).astype(fp8)
    csq_bc = np.ascontiguousarray(
        np.broadcast_to((csq * np.float32(inv_a)).astype(np.float32)[None, :], (P, K))
    )
    return embT8, biasR, cenT8, csq8, csq_bc


def _get_jitted(alpha: float):
    key = (float(alpha), os.environ.get("KBENCH_REPS", "1"),
           os.environ.get("KOPT_WARM", "24"),
           os.environ.get("KOPT_SKIP", ""), os.environ.get("KOPT_DR", "1"),
           os.environ.get("KOPT_FOLD", "dve"), os.environ.get("KOPT_MUL", "dve"),
           os.environ.get("KOPT_GBLK", str(G_BLK)),
           os.environ.get("KOPT_RS", "block"),
           os.environ.get("KOPT_ACTB", "0"), str(S_BLK),
           os.environ.get("KOPT_PS2", "0"), os.environ.get("KOPT_INV", "dve"))
    if key in _CACHE:
        return _CACHE[key]

    import jax
    from jax.experimental.shard_map import shard_map
    from jax.sharding import Mesh, PartitionSpec as PS

    import concourse.mybir as mybir
    import concourse.tile as tile
    from concourse.bass2jax import bass_jit

    in_specs = (PS(None, "core"), PS(None, "core"), PS(None), PS(None), PS(None))

    bf16 = mybir.dt.bfloat16

    def body(nc, embT8, biasR, cenT8, csq8, csq_bc):
        out_d = nc.dram_tensor(
            "cluster_p", [NC_ROWS, K], bf16, kind="ExternalOutput"
        )
        with tile.TileContext(nc) as tc:
            _emit(nc, tc, embT8, biasR, cenT8, csq8, csq_bc, out_d,
                  float(alpha), NC_ROWS)
        return out_d

    f = bass_jit(body, num_devices=N_CORES)
    mesh = Mesh(np.asarray(jax.devices()[:N_CORES]), ("core",))
    sharded = shard_map(
        f,
        mesh=mesh,
        in_specs=in_specs,
        out_specs=PS("core"),
        check_rep=False,
    )
    jitted = jax.jit(sharded)
    _CACHE[key] = (jitted, mesh)
    return _CACHE[key]


def kernel(embeddings, cluster_centers, alpha):
    alpha = float(alpha)
    args = _host_prep(embeddings, cluster_centers, alpha)
    jitted, _ = _get_jitted(alpha)
    try:
        out = jitted(*args)
        return np.asarray(out).astype(np.float32)
    except Exception:
        # transient device hiccups have been observed; retry once
        import time as _time

        _time.sleep(60)
        out = jitted(*args)
        return np.asarray(out).astype(np.float32)


def _build_program(alpha: float):
    """Standalone Bacc program (for CoreSim checks)."""
    import concourse.bacc as bacc
    import concourse.mybir as mybir
    import concourse.tile as tile

    f32 = mybir.dt.float32
    fp8 = mybir.dt.float8e4
    bf16 = mybir.dt.bfloat16
    nc = bacc.Bacc(None, target_bir_lowering=False, debug=False, num_devices=N_CORES)
    embT_d = nc.declare_dram_parameter("embT8", [D, NC_ROWS], fp8, isOutput=False)
    bias_d = nc.declare_dram_parameter("biasR", [P, NC_BLKS], f32, isOutput=False)
    cen_d = nc.declare_dram_parameter("cenT8", [D, K], fp8, isOutput=False)
    csq_d = nc.declare_dram_parameter("csq8", [P, 2, K + P], fp8, isOutput=False)
    csqbc_d = nc.declare_dram_parameter("csq_bc", [P, K], f32, isOutput=False)
    out_d = nc.declare_dram_parameter("cluster_p", [NC_ROWS, K], bf16, isOutput=True)
    with tile.TileContext(nc) as tc:
        _emit(nc, tc, embT_d, bias_d, cen_d, csq_d, csqbc_d, out_d, alpha, NC_ROWS)
    nc.finalize()
    return nc


# revision 66
# speedup vs baseline: 1.0496x; 1.0438x over previous
"""DEC soft-assignment (student-t, row-normalized) Trainium2 Bass kernel.

q[n,k] = (1 + ||x_n - c_k||^2/alpha)^(-(alpha+1)/2), row-normalized.

Strategy (8 cores, data-parallel over N), v2 (~52 us/core steady state,
median of 6 samples with +-5 us run noise, vs the 149.6 us v1 baseline):
  ||x-c||^2 = ||x||^2 - 2 x.c + ||c||^2 expanded on-chip; the O(ND)
  norms and data-layout prep are done host-side so the device program is
  a pure fp8 DoubleRow matmul pipeline near the compute roofline:
  - host ships embT8 [D, N/8] fp8e4 (pre-transposed: d on partitions, so
    no PE transposes on device), cenT8 [D, K] fp8e4 scaled by -2/alpha,
    biasR = 1 + ||x||^2/alpha (f32, per-row ACT bias), and csq_bc
    [128, K] f32 = ||c||^2/alpha broadcast.
  - per 128-row block: PSUM <- 3 DoubleRow fp8 matmuls (256-contraction
    each, moving cenT 512 cols); DVE adds csq_bc (a PE fold matmul
    measured ~2x slower than this tensor_tensor add); ONE ACT Reciprocal
    computes numer = 1/(psum + biasR[p]) -> bf16 AND the row-sum via
    accum_out; DVE does 1/rowsum + normalize into a bf16 output tile.
  - output is bf16 (0.4% quantization << 2e-2 gate), halving write
    traffic; host upcasts to f32.  Measured rel err 1.05e-2.
  - measured engine budget per block: PE ~660ns (3 MMs), DVE ~925ns
    (add+recip+mul, the binding engine), ACT ~570ns, DMA ~45us total.
  - queue/overlap details: setup loads ride the SWDGE queue so the sync
    queue carries only embedding tiles; the first prefetch is split in
    half so compute starts early; outputs ship per half-supertile for
    tail overlap.  Moving the tiny 1/rowsum to ACT measured WORSE
    (cross-engine coupling per block) - it stays on DVE.
  KOPT_* env knobs switch the A/B variants explored during tuning (all
  defaults encode the fastest measured configuration).
"""

import contextlib
import os
import sys

sys.path.insert(0, "/opt/trn_rl_repo")

import numpy as np

N_CORES = 8
N, D, K = 65536, 768, 512
NC_ROWS = N // N_CORES          # 8192 rows per core
P = 128                         # partitions
S_BLK = int(os.environ.get("KOPT_SBLK", "8"))  # 128-row blocks per supertile
S_ROWS = P * S_BLK              # rows per supertile
N_SUPER = NC_ROWS // S_ROWS     # supertiles per core
NC_BLKS = NC_ROWS // P          # 64 blocks per core
D_CHUNKS = D // P               # 6 contraction chunks
G_BLK = 4                       # blocks per fold/recip group

_CACHE = {}


def _emit(nc, tc, embT_d, bias_d, cen_d, csq_d, csqbc_d, out_d, alpha: float, n_rows: int):
    """Emit the per-core program into an open TileContext."""
    import concourse.bass as bass
    import concourse.mybir as mybir

    f32 = mybir.dt.float32
    bf16 = mybir.dt.bfloat16
    fp8 = mybir.dt.float8e4
    DR = mybir.MatmulPerfMode.DoubleRow

    power = (alpha + 1.0) / 2.0
    n_super = n_rows // S_ROWS
    n_blks = n_rows // P
    reps = int(os.environ.get("KBENCH_REPS", "1"))
    # perturbation-profiling knobs (bottleneck attribution without NTFF):
    # comma-set of {mm,act,dve,out} to skip
    skip = set(filter(None, os.environ.get("KOPT_SKIP", "").split(",")))
    use_dr = os.environ.get("KOPT_DR", "1") == "1"
    fold_mode = os.environ.get("KOPT_FOLD", "dve")  # pe | dve | preload
    mul_mode = os.environ.get("KOPT_MUL", "dve")    # dve | gps | split
    g_blk = int(os.environ.get("KOPT_GBLK", str(G_BLK)))
    rs_mode = os.environ.get("KOPT_RS", "block")    # block | batch
    # csq as a full elementwise bias AP on the recip ACT (hw experiment);
    # xsq then moves to a 2x-mode tensor_scalar add on DVE
    act_bias = os.environ.get("KOPT_ACTB", "0") == "1"
    ps2 = os.environ.get("KOPT_PS2", "0") == "1"    # 2-bank psum pairs
    inv_eng = os.environ.get("KOPT_INV", "dve")     # dve | act
    psum_bufs = max(6, g_blk)

    emb_v = embT_d.rearrange("(c p) (s n) -> s p c n", p=P, c=D_CHUNKS, n=S_ROWS)
    out_v = out_d.rearrange("(s a p) k -> s p a k", p=P, a=S_BLK)

    def act_recip(out_ap, in_ap, bias_ap, accum_ap):
        """numer = 1/(in + bias[p]); accum_out = row-sum(numer).
        Direct InstActivation(Reciprocal): measured max rel err ~1.2e-5
        on hw, well within this problem's 2e-2 gate."""
        eng = nc.scalar
        ins = [
            eng.lower_ap(in_ap),
            eng.lower_ap(bias_ap),
            mybir.ImmediateValue(dtype=f32, value=1.0),
            mybir.ImmediateValue(dtype=f32, value=0.0),
        ]
        outs = [eng.lower_ap(out_ap), eng.lower_ap(accum_ap)]
        return eng.add_instruction(
            mybir.InstActivation(
                name=nc.get_next_instruction_name(),
                func=mybir.ActivationFunctionType.Reciprocal,
                ins=ins,
                outs=outs,
            )
        )

    def act_recip_simple(out_ap, in_ap):
        """out = 1/in on the ACT engine (it has slack; DVE is binding)."""
        eng = nc.scalar
        ins = [
            eng.lower_ap(in_ap),
            mybir.ImmediateValue(dtype=f32, value=0.0),
            mybir.ImmediateValue(dtype=f32, value=1.0),
            mybir.ImmediateValue(dtype=f32, value=0.0),
        ]
        return eng.add_instruction(
            mybir.InstActivation(
                name=nc.get_next_instruction_name(),
                func=mybir.ActivationFunctionType.Reciprocal,
                ins=ins,
                outs=[eng.lower_ap(out_ap)],
            )
        )

    with contextlib.ExitStack() as stack:
        const_pool = stack.enter_context(tc.tile_pool(name="const", bufs=1))
        in_pool = stack.enter_context(tc.tile_pool(name="io_in", bufs=3))

        # resident operands (small DMAs, issued first)
        cenT = const_pool.tile([P, D_CHUNKS, K], fp8, name="cenT")
        csq8 = const_pool.tile([P, 2, K + P], fp8, name="csq8")
        biasR = const_pool.tile([P, n_blks], f32, name="biasR")
        csq_bc = const_pool.tile([P, K], f32, name="csq_bc")
        # setup loads ride the (idle) SWDGE queue so the first embedding
        # prefetches own the sync/HWDGE queue from instruction 0
        nc.gpsimd.dma_start(cenT[:], cen_d.rearrange("(c p) k -> p c k", p=P))
        nc.gpsimd.dma_start(csq8[:], csq_d[:])
        nc.gpsimd.dma_start(biasR[:], bias_d[:])
        if fold_mode in ("dve", "preload"):
            nc.gpsimd.dma_start(csq_bc[:], csqbc_d[:])
        # contiguous, aligned copies of the fold operands (the strided
        # [*,*,640] slices measure ~2x slower as matmul operands)
        ones_t = const_pool.tile([P, 2, P], fp8, name="ones_t")
        csq_mv_t = const_pool.tile([P, 2, K], fp8, name="csq_mv_t")
        nc.scalar.copy(ones_t[:], csq8[:, :, K : K + P])
        nc.scalar.copy(csq_mv_t[:], csq8[:, :, 0:K])
        ones_st = ones_t[:]     # DoubleRow fold stationary
        csq_mv = csq_mv_t[:]    # DoubleRow fold moving

        # prefetch the first supertiles' embeddings during setup; the very
        # first half-tile gets its own DMA so block 0 compute can start
        # after ~half the transfer time
        prefetched = {}
        for i in range(min(2, n_super)):
            t_in = in_pool.tile([P, D_CHUNKS, S_ROWS], fp8, tag="emb")
            if i == 0:
                half = S_ROWS // 2
                nc.sync.dma_start(t_in[:, :, 0:half], emb_v[i][:, :, 0:half])
                nc.sync.dma_start(t_in[:, :, half:], emb_v[i][:, :, half:])
            else:
                nc.sync.dma_start(t_in[:], emb_v[i])
            prefetched[i] = t_in

        # PE warmup: dense burst of dummy matmuls while the first DMAs
        # land, ramping the tensor engine's HAM clock-gate to full rate.
        n_warm = int(os.environ.get("KOPT_WARM", "24"))
        if n_warm:
            with (
                tc.tile_pool(name="warm", bufs=1) as warm_pool,
                tc.tile_pool(name="warm_ps", bufs=1, space=bass.MemorySpace.PSUM) as warm_ps_pool,
            ):
                wz = warm_pool.tile([P, P], f32)
                nc.gpsimd.memset(wz[:], 0.0)
                warm_ps = warm_ps_pool.tile([P, P], f32, tag="warm")
                for _ in range(n_warm):
                    nc.tensor.matmul(warm_ps[:], wz[:], wz[:], start=True, stop=True)

        ps_seq = [0]  # running count of ps allocations (first-use detection)
        with (
            tc.tile_pool(name="blk", bufs=8) as blk_pool,
            tc.tile_pool(name="io_out", bufs=2) as out_pool,
            tc.tile_pool(name="mm_ps", bufs=psum_bufs, space=bass.MemorySpace.PSUM) as mm_ps,
        ):
            for i in [t for _ in range(reps) for t in range(n_super)]:
                emb_t = prefetched.pop(i, None)
                if emb_t is None:
                    emb_t = in_pool.tile([P, D_CHUNKS, S_ROWS], fp8, tag="emb")
                    nc.sync.dma_start(emb_t[:], emb_v[i])

                out_t = out_pool.tile([P, S_BLK, K], bf16, tag="out")

                for g0 in range(0, S_BLK, g_blk):
                    blocks = range(g0, g0 + g_blk)
                    ps = {}
                    numers = {}
                    rs_g = None
                    # csq folds: ones stationary loaded once per group
                    skip_fold = ("mm" in skip or "fold" in skip
                                 or fold_mode not in ("pe", "pe_inline"))
                    skip_cross = "mm" in skip or "cross" in skip
                    preload = {}
                    pair_t = {}
                    for b in blocks:
                        if ps2:
                            if b % 2 == 0:
                                p2 = mm_ps.tile([P, 2, K], f32, tag="cross2",
                                                name=f"ps2_{b}", bufs=3)
                                pair_t[b] = pair_t[b + 1] = p2
                            ps[b] = pair_t[b][:, b % 2, :]
                        else:
                            t = mm_ps.tile([P, K], f32, tag="cross",
                                           name=f"ps{b}")
                            ps[b] = t[:]
                        # preload mode: after each PSUM buffer's first use,
                        # has_written is all-set, so a DVE copy of the csq row
                        # followed by start=False matmuls accumulates on top.
                        preload[b] = (
                            fold_mode == "preload"
                            and "mm" not in skip
                            and ps_seq[0] >= psum_bufs
                        )
                        ps_seq[0] += 1
                        if skip_fold:
                            if preload[b]:
                                nc.vector.tensor_copy(out=ps[b], in_=csq_bc[:])
                            continue
                        if fold_mode != "pe_inline":
                            nc.tensor.matmul(
                                ps[b], ones_st, csq_mv,
                                start=True, stop=skip_cross, perf_mode=DR,
                            )
                    for b in blocks:
                        # PSUM += -(2/a) x.c  (3 DoubleRow fp8 matmuls)
                        if fold_mode == "pe_inline" and not skip_fold:
                            nc.tensor.matmul(
                                ps[b], ones_st, csq_mv,
                                start=True, stop=skip_cross, perf_mode=DR,
                            )
                        if not skip_cross:
                            if use_dr:
                                for j in range(0, D_CHUNKS, 2):
                                    nc.tensor.matmul(
                                        ps[b],
                                        emb_t[:, j : j + 2, b * P : (b + 1) * P],
                                        cenT[:, j : j + 2, :],
                                        start=(skip_fold and j == 0
                                               and not preload[b]),
                                        stop=(j == D_CHUNKS - 2),
                                        perf_mode=DR,
                                        skip_group_check=preload[b],
                                    )
                            else:
                                for j in range(D_CHUNKS):
                                    nc.tensor.matmul(
                                        ps[b],
                                        emb_t[:, j, b * P : (b + 1) * P],
                                        cenT[:, j, :],
                                        start=(skip_fold and j == 0
                                               and not preload[b]),
                                        stop=(j == D_CHUNKS - 1),
                                        skip_group_check=preload[b],
                                    )
                        gb = i * S_BLK + b  # global block index (bias col)
                        if "act" in skip:
                            continue
                        do_fold_add = (
                            (fold_mode == "dve" or
                             (fold_mode == "preload" and not preload[b]))
                            and "mm" not in skip
                        )
                        todo = [b]
                        if ps2 and power == 1.0 and rs_mode == "block":
                            if b % 2 == 0:
                                continue  # processed with the odd partner
                            if do_fold_add:
                                nc.vector.tensor_tensor(
                                    pair_t[b][:, :, :], pair_t[b][:, :, :],
                                    csq_bc[:, None, :].to_broadcast([P, 2, K]),
                                    mybir.AluOpType.add,
                                )
                            todo = [b - 1, b]
                        elif do_fold_add:
                            if act_bias:
                                # xsq via 2x-mode tensor_scalar (csq rides the
                                # ACT bias slot elementwise)
                                nc.vector.tensor_scalar_add(
                                    ps[b], ps[b],
                                    biasR[:, gb : gb + 1],
                                )
                            else:
                                # csq add on DVE instead of the PE fold matmul
                                nc.vector.tensor_tensor(
                                    ps[b], ps[b], csq_bc[:],
                                    mybir.AluOpType.add,
                                )
                        if power == 1.0:
                            for bb in todo:
                                gbb = i * S_BLK + bb
                                # numer = 1/(ps + bias) -> bf16 + rowsum, one ACT
                                numer = blk_pool.tile([P, K], bf16, tag="numer",
                                                      bufs=g_blk + 4)
                                bias_ap = (csq_bc[:] if act_bias
                                           else biasR[:, gbb : gbb + 1])
                                if rs_mode == "batch":
                                    if rs_g is None:
                                        rs_g = blk_pool.tile([P, g_blk], f32,
                                                             tag="rs")
                                    act_recip(
                                        numer[:], ps[bb], bias_ap,
                                        rs_g[:, bb - g0 : bb - g0 + 1],
                                    )
                                    numers[bb] = numer
                                else:
                                    rs = blk_pool.tile([P, 1], f32, tag="rs1")
                                    act_recip(
                                        numer[:], ps[bb], bias_ap,
                                        rs[:],
                                    )
                                    if "dve" in skip:
                                        continue
                                    inv = blk_pool.tile([P, 1], f32, tag="inv1")
                                    if inv_eng == "act":
                                        act_recip_simple(inv[:], rs[:])
                                    else:
                                        nc.vector.reciprocal(inv[:], rs[:])
                                    nc.vector.tensor_scalar_mul(
                                        out_t[:, bb, :], numer[:], inv[:]
                                    )
                        else:
                            denom = blk_pool.tile([P, K], f32, tag="denom")
                            nc.scalar.activation(
                                denom[:], ps[b],
                                mybir.ActivationFunctionType.Identity,
                                bias=biasR[:, gb : gb + 1], scale=1.0,
                            )
                            lnd = blk_pool.tile([P, K], f32, tag="lnd")
                            nc.scalar.activation(
                                lnd[:], denom[:], mybir.ActivationFunctionType.Ln
                            )
                            numer = blk_pool.tile([P, K], bf16, tag="numer")
                            rs = blk_pool.tile([P, 1], f32, tag="rs")
                            nc.scalar.activation(
                                numer[:], lnd[:],
                                mybir.ActivationFunctionType.Exp,
                                scale=-power, accum_out=rs[:],
                            )
                            inv = blk_pool.tile([P, 1], f32, tag="inv")
                            nc.vector.reciprocal(inv[:], rs[:])
                            nc.vector.tensor_scalar_mul(
                                out_t[:, b, :], numer[:], inv[:]
                            )

                    if power == 1.0 and numers and "dve" not in skip:
                        # batched 1/rowsum for the group, then normalize
                        inv_g = blk_pool.tile([P, g_blk], f32, tag="inv")
                        nc.vector.reciprocal(inv_g[:], rs_g[:])
                        for b in blocks:
                            if b not in numers:
                                continue
                            sc = inv_g[:, b - g0 : b - g0 + 1]
                            if mul_mode == "split" and (b % 2 == 1):
                                nc.scalar.activation(
                                    out_t[:, b, :], numers[b][:],
                                    mybir.ActivationFunctionType.Copy,
                                    scale=sc,
                                )
                            elif mul_mode == "gps":
                                nc.gpsimd.tensor_scalar_mul(
                                    out_t[:, b, :], numers[b][:], sc
                                )
                            else:
                                nc.vector.tensor_scalar_mul(
                                    out_t[:, b, :], numers[b][:], sc
                                )

                # output triggers on the (otherwise idle) GpSimd queue; two
                # half-supertile DMAs so the first half ships while the
                # second half computes (subtile deps), shortening the tail
                if "out" not in skip:
                    h = S_BLK // 2
                    nc.gpsimd.dma_start(out_v[i][:, 0:h, :], out_t[:, 0:h, :])
                    nc.gpsimd.dma_start(out_v[i][:, h:, :], out_t[:, h:, :])


def _host_prep(embeddings, cluster_centers, alpha: float):
    """Layout/precision prep: transpose+fp8-cast emb, row norms, center
    norms (hi/lo fp8 split), -2/alpha-scaled fp8 centers-T."""
    import ml_dtypes

    fp8 = ml_dtypes.float8_e4m3
    bf16 = ml_dtypes.bfloat16  # noqa: F841  (output dtype, upcast in kernel())

    emb = np.ascontiguousarray(np.asarray(embeddings, dtype=np.float32))
    cen = np.ascontiguousarray(np.asarray(cluster_centers, dtype=np.float32))
    inv_a = 1.0 / alpha

    embT8 = np.ascontiguousarray(emb.astype(fp8).T)              # [D, N]
    xsq = np.einsum("nd,nd->n", emb, emb, dtype=np.float32)
    biasR = np.ascontiguousarray(
        (1.0 + xsq * inv_a).astype(np.float32).reshape(N // P, P).T
    )                                                            # [P, N/P]
    cenT8 = np.ascontiguousarray((cen.T * np.float32(-2.0 * inv_a)).astype(fp8))
    csq = np.einsum("kd,kd->k", cen, cen, dtype=np.float32)
    tgt = (csq * np.float32(inv_a / P)).astype(np.float32)       # [K]
    hi = tgt.astype(fp8)
    lo = (tgt - hi.astype(np.float32)).astype(fp8)
    csq8 = np.empty((P, 2, K + P), dtype=fp8)
    csq8[:, 0, :K] = hi[None, :]
    csq8[:, 1, :K] = lo[None, :]
    csq8[:, :, K:] = np.float32(1.0).astype(fp8)
    csq_bc = np.ascontiguousarray(
        np.broadcast_to((csq * np.float32(inv_a)).astype(np.float32)[None, :], (P, K))
    )
    return embT8, biasR, cenT8, csq8, csq_bc


def _get_jitted(alpha: float):
    key = (float(alpha), os.environ.get("KBENCH_REPS", "1"),
           os.environ.get("KOPT_WARM", "24"),
           os.environ.get("KOPT_SKIP", ""), os.environ.get("KOPT_DR", "1"),
           os.environ.get("KOPT_FOLD", "dve"), os.environ.get("KOPT_MUL", "dve"),
           os.environ.get("KOPT_GBLK", str(G_BLK)),
           os.environ.get("KOPT_RS", "block"),
           os.environ.get("KOPT_ACTB", "0"), str(S_BLK),
           os.environ.get("KOPT_PS2", "0"), os.environ.get("KOPT_INV", "dve"))
    if key in _CACHE:
        return _CACHE[key]

    import jax
    from jax.experimental.shard_map import shard_map
    from jax.sharding import Mesh, PartitionSpec as PS

    import concourse.mybir as mybir
    import concourse.tile as tile
    from concourse.bass2jax import bass_jit

    in_specs = (PS(None, "core"), PS(None, "core"), PS(None), PS(None), PS(None))

    bf16 = mybir.dt.bfloat16

    def body(nc, embT8, biasR, cenT8, csq8, csq_bc):
        out_d = nc.dram_tensor(
            "cluster_p", [NC_ROWS, K], bf16, kind="ExternalOutput"
        )
        with tile.TileContext(nc) as tc:
            _emit(nc, tc, embT8, biasR, cenT8, csq8, csq_bc, out_d,
                  float(alpha), NC_ROWS)
        return out_d

    f = bass_jit(body, num_devices=N_CORES)
    mesh = Mesh(np.asarray(jax.devices()[:N_CORES]), ("core",))
    sharded = shard_map(
        f,
        mesh=mesh,
        in_specs=in_specs,
        out_specs=PS("core"),
        check_rep=False,
    )
    jitted = jax.jit(sharded)
    _CACHE[key] = (jitted, mesh)
    return _CACHE[key]


def kernel(embeddings, cluster_centers, alpha):
    alpha = float(alpha)
    args = _host_prep(embeddings, cluster_centers, alpha)
    jitted, _ = _get_jitted(alpha)
    try:
        out = jitted(*args)
        return np.asarray(out).astype(np.float32)
    except Exception:
        # transient device hiccups have been observed; retry once
        import time as _time

        _time.sleep(60)
        out = jitted(*args)
        return np.asarray(out).astype(np.float32)


def _build_program(alpha: float):
    """Standalone Bacc program (for CoreSim checks)."""
    import concourse.bacc as bacc
    import concourse.mybir as mybir
    import concourse.tile as tile

    f32 = mybir.dt.float32
    fp8 = mybir.dt.float8e4
    bf16 = mybir.dt.bfloat16
    nc = bacc.Bacc(None, target_bir_lowering=False, debug=False, num_devices=N_CORES)
    embT_d = nc.declare_dram_parameter("embT8", [D, NC_ROWS], fp8, isOutput=False)
    bias_d = nc.declare_dram_parameter("biasR", [P, NC_BLKS], f32, isOutput=False)
    cen_d = nc.declare_dram_parameter("cenT8", [D, K], fp8, isOutput=False)
    csq_d = nc.declare_dram_parameter("csq8", [P, 2, K + P], fp8, isOutput=False)
    csqbc_d = nc.declare_dram_parameter("csq_bc", [P, K], f32, isOutput=False)
    out_d = nc.declare_dram_parameter("cluster_p", [NC_ROWS, K], bf16, isOutput=True)
    with tile.TileContext(nc) as tc:
        _emit(nc, tc, embT_d, bias_d, cen_d, csq_d, csqbc_d, out_d, alpha, NC_ROWS)
    nc.finalize()
    return nc
